# revision 8
# baseline (speedup 1.0000x reference)
"""Trainium2 Bass kernel for nn_CCL_50740743635433 (class-collapsed CCL loss).

Math: with C=64 classes, pos_centroid[i] == class_centroid[labels[i]], so the
reference's 8192x8192 distance matrix collapses to 8192x64:
  class_sum[c,:]  = sum_{i: lab=i==c} preds[i,:]      (one-hot matmul)
  cent[c,:]       = class_sum[c,:] / count[c]
  sq[i,c]         = relu(|p_i|^2 + |cent_c|^2 - 2 p_i.cent_c)
  pos[i]          = sqrt(sq[i, lab_i]);  neg[i] = sqrt(min_{c != lab_i} sq[i,c])
  loss            = mean softplus(pos - neg + 0.2)

Distribution (8 cores, no collectives): every core receives the FULL preds and
computes the class sums redundantly (cross-core collectives cost ~70us on this
rig vs ~17us of local compute); each core then evaluates distances + softplus
only for its own 1024-row shard and returns a partial sum; the host adds the 8
partials and divides by N.
"""

import sys

sys.path.insert(0, "/opt/trn_rl_repo")

import numpy as np

import concourse.bacc as bacc
import concourse.bass_utils as bass_utils
import concourse.mybir as mybir
import concourse.tile as tile

N = 8192
D = 128
C = 64
N_CORES = 8
ROWS_PER_CORE = N // N_CORES          # 1024
CHUNKS = N // 128                     # 64 chunks of 128 rows
OWN_CHUNKS = ROWS_PER_CORE // 128     # 8 chunks per core
ALPHA = 0.2
BIG = 1e10
HUGE = 1e20
FMAX = 3.0e38

f32 = mybir.dt.float32
Alu = mybir.AluOpType
Act = mybir.ActivationFunctionType

_compiled = None
last_results = None


def _build():
    nc = bacc.Bacc(
        "TRN2",
        target_bir_lowering=False,
        debug=False,
        enable_asserts=True,
        num_devices=N_CORES,
    )

    preds_d = nc.dram_tensor("preds", [N, D], f32, kind="ExternalInput")
    labels_d = nc.dram_tensor("labels", [128, CHUNKS], f32, kind="ExternalInput")
    mypreds_d = nc.dram_tensor("my_preds", [ROWS_PER_CORE, D], f32, kind="ExternalInput")
    mylab_d = nc.dram_tensor("my_labels", [128, OWN_CHUNKS], f32, kind="ExternalInput")
    out_d = nc.dram_tensor("out", [1, 1], f32, kind="ExternalOutput")

    iota_d = nc.inline_tensor(
        np.tile(np.arange(C, dtype=np.float32), (128, 1)), name="iota64"
    )
    ident_d = nc.inline_tensor(np.eye(128, dtype=np.float32), name="ident128")
    onesc_d = nc.inline_tensor(np.ones((128, 1), dtype=np.float32), name="ones_col")
    onesr_d = nc.inline_tensor(np.ones((1, 128), dtype=np.float32), name="ones_row")

    with tile.TileContext(nc) as tc:
        with (
            tc.tile_pool(name="cst", bufs=1) as cst,
            tc.tile_pool(name="big", bufs=1) as bigp,
            tc.tile_pool(name="wrk", bufs=1) as wrk,
            tc.tile_pool(name="ohp", bufs=4) as ohp,
            tc.tile_pool(name="scr", bufs=2) as scr,
            tc.tile_pool(name="pacc", bufs=1, space="PSUM") as pacc,
            tc.tile_pool(name="pt", bufs=2, space="PSUM") as pt,
            tc.tile_pool(name="pg", bufs=2, space="PSUM") as pg,
            tc.tile_pool(name="psm", bufs=2, space="PSUM") as psm,
        ):
            # ---- constants / inputs to SBUF ----
            iota_sb = cst.tile([128, C], f32)
            nc.sync.dma_start(iota_sb[:], iota_d.ap())
            ident_sb = cst.tile([128, 128], f32)
            nc.sync.dma_start(ident_sb[:], ident_d.ap())
            onesc_sb = cst.tile([128, 1], f32)
            nc.sync.dma_start(onesc_sb[:], onesc_d.ap())
            onesr_sb = cst.tile([1, 128], f32)
            nc.sync.dma_start(onesr_sb[:], onesr_d.ap())

            alpha_sb = cst.tile([128, 1], f32)
            nc.vector.memset(alpha_sb[:], ALPHA)

            lsb = cst.tile([128, CHUNKS], f32)
            nc.sync.dma_start(lsb[:], labels_d.ap())
            mylsb = cst.tile([128, OWN_CHUNKS], f32)
            nc.sync.dma_start(mylsb[:], mylab_d.ap())

            # full preds, chunk-major: psb[p, j, d] = preds[j*128 + p, d];
            # column D holds 1.0 so one matmul also accumulates class counts.
            psb = bigp.tile([128, CHUNKS, D + 1], f32)
            preds_re = preds_d.ap().rearrange("(j p) d -> p j d", p=128)
            DMA_SPLIT = 8
            step = CHUNKS // DMA_SPLIT
            for s in range(DMA_SPLIT):
                nc.sync.dma_start(
                    psb[:, s * step : (s + 1) * step, 0:D],
                    preds_re[:, s * step : (s + 1) * step, :],
                )
            nc.vector.memset(psb[:, :, D : D + 1], 1.0)

            # own shard, chunk-major
            osb = wrk.tile([128, OWN_CHUNKS, D], f32)
            nc.sync.dma_start(
                osb[:], mypreds_d.ap().rearrange("(j p) d -> p j d", p=128)
            )

            # ---- phase A: class sums + counts via one-hot matmuls ----
            # psum_cs[c, 0:D] = sum_i BIG*oh[i,c]*preds[i,:], [c, D] = BIG*count
            psum_cs = pacc.tile([C, D + 1], f32)
            for j in range(CHUNKS):
                ohj = ohp.tile([128, C], f32, name=f"oh{j}", tag="oh")
                nc.vector.tensor_scalar(
                    ohj[:], iota_sb[:], lsb[:, j : j + 1], BIG, Alu.is_equal, Alu.mult
                )
                nc.tensor.matmul(
                    psum_cs[:],
                    ohj[:],
                    psb[:, j, :],
                    start=(j == 0),
                    stop=(j == CHUNKS - 1),
                )

            # ---- own-shard prep (independent of phase A results) ----
            # masks for own chunks
            ohR = wrk.tile([128, OWN_CHUNKS, C], f32)
            invR = wrk.tile([128, OWN_CHUNKS, C], f32)
            for j in range(OWN_CHUNKS):
                nc.vector.tensor_scalar(
                    ohR[:, j, :], iota_sb[:], mylsb[:, j : j + 1], BIG,
                    Alu.is_equal, Alu.mult,
                )
                nc.vector.tensor_scalar(
                    invR[:, j, :], iota_sb[:], mylsb[:, j : j + 1], BIG,
                    Alu.not_equal, Alu.mult,
                )
            # p_sq per own row, and -2*preds^T chunks for the G matmul
            psq = wrk.tile([128, OWN_CHUNKS], f32)
            pts = wrk.tile([128, OWN_CHUNKS, D], f32)
            for j in range(OWN_CHUNKS):
                sqscr = scr.tile([128, D], f32, name=f"sqscr{j}", tag="sqscr")
                nc.scalar.activation(
                    sqscr[:], osb[:, j, :], Act.Square,
                    accum_out=psq[:, j : j + 1],
                )
                ptp = pt.tile([128, 128], f32, name=f"ptp{j}", tag="ptp")
                nc.tensor.transpose(ptp[:], osb[:, j, :], ident_sb[:])
                nc.scalar.activation(pts[:, j, :], ptp[:], Act.Copy, scale=-2.0)

            # ---- centroids ----
            cs_sb = wrk.tile([C, D + 1], f32)
            nc.scalar.activation(cs_sb[:], psum_cs[:], Act.Copy)
            # rcat cols: [1/max(cnt,1) | 1e20*(cnt==0)] (column space, base 0)
            rcat = wrk.tile([C, 2], f32)
            safe = wrk.tile([C, 1], f32)
            nc.vector.tensor_scalar(
                safe[:], cs_sb[:, D : D + 1], 1.0, None, Alu.max
            )
            nc.vector.reciprocal(rcat[:, 0:1], safe[:])
            nc.vector.tensor_scalar(
                rcat[:, 1:2], cs_sb[:, D : D + 1], 0.0, HUGE,
                Alu.is_equal, Alu.mult,
            )
            # transpose each column to a row (both land at partition 0)
            psum_rt = psm.tile([1, C], f32, name="psum_rt", tag="sm")
            nc.tensor.matmul(psum_rt[:], rcat[:, 0:1], ident_sb[0:C, 0:C])
            rrow = wrk.tile([1, C], f32)
            nc.scalar.activation(rrow[:], psum_rt[:], Act.Copy)
            psum_ab = psm.tile([1, C], f32, name="psum_ab", tag="sm")
            nc.tensor.matmul(psum_ab[:], rcat[:, 1:2], ident_sb[0:C, 0:C])
            ab_sb = wrk.tile([1, C], f32)
            nc.scalar.activation(ab_sb[:], psum_ab[:], Act.Copy)

            # centT[d, c] = class_sum[c, d] * recip[c]
            psum_ct = pt.tile([128, C], f32, name="psum_ct", tag="ptp")
            nc.tensor.transpose(psum_ct[:], cs_sb[:, 0:D], ident_sb[0:C, 0:C])
            ctsb = wrk.tile([128, C], f32)
            nc.scalar.activation(ctsb[:], psum_ct[:], Act.Copy)
            psum_rb = psm.tile([128, C], f32, name="psum_rb", tag="sm")
            nc.tensor.matmul(psum_rb[:], onesr_sb[:], rrow[:])
            centT = wrk.tile([128, C], f32)
            nc.vector.tensor_tensor(centT[:], ctsb[:], psum_rb[:], Alu.mult)

            # c_sq row (+1e20 on absent classes), broadcast to all partitions
            sqc = wrk.tile([128, C], f32)
            nc.vector.tensor_tensor(sqc[:], centT[:], centT[:], Alu.mult)
            psum_csq = psm.tile([1, C], f32, name="psum_csq", tag="sm")
            nc.tensor.matmul(psum_csq[:], onesc_sb[:], sqc[:])
            csqr = wrk.tile([1, C], f32)
            nc.vector.tensor_tensor(csqr[:], psum_csq[:], ab_sb[:], Alu.add)
            psum_cb = psm.tile([128, C], f32, name="psum_cb", tag="sm")
            nc.tensor.matmul(psum_cb[:], onesr_sb[:], csqr[:])
            csq_sb = wrk.tile([128, C], f32)
            nc.scalar.activation(csq_sb[:], psum_cb[:], Act.Copy)

            # ---- phase F: per own chunk distances, masked mins ----
            # pnsq even cols = negsq (min over other classes of sq),
            # odd cols = possq (sq at own class)
            pnsq = wrk.tile([128, 2 * OWN_CHUNKS], f32)
            for j in range(OWN_CHUNKS):
                psum_g = pg.tile([128, C], f32, name=f"psum_g{j}", tag="g")
                nc.tensor.matmul(psum_g[:], pts[:, j, :], centT[:])
                hc = scr.tile([128, C], f32, name=f"hc{j}", tag="hc")
                nc.vector.tensor_tensor(hc[:], psum_g[:], csq_sb[:], Alu.add)
                sqj = scr.tile([128, C], f32, name=f"sqj{j}", tag="sqj")
                nc.scalar.activation(
                    sqj[:], hc[:], Act.Relu, bias=psq[:, j : j + 1]
                )
                pair = scr.tile([128, 2, C], f32, name=f"pair{j}", tag="pair")
                nc.vector.tensor_tensor(
                    pair[:, 0, :], sqj[:], ohR[:, j, :], Alu.add
                )
                nc.vector.tensor_tensor(
                    pair[:, 1, :], sqj[:], invR[:, j, :], Alu.add
                )
                nc.vector.tensor_reduce(
                    pnsq[:, 2 * j : 2 * j + 2], pair[:],
                    mybir.AxisListType.X, Alu.min,
                )

            # ---- tail: sqrt via Newton rsqrt on DVE (keeps ACT on one
            # table set: Copy/Square/Relu/Exp/Ln), then softplus ----
            W = 2 * OWN_CHUNKS
            i32 = mybir.dt.int32
            z = wrk.tile([128, W], f32)
            tsh = wrk.tile([128, W], f32)
            nc.vector.tensor_scalar(
                tsh[:].bitcast(i32), pnsq[:].bitcast(i32), 1, None,
                Alu.logical_shift_right,
            )
            nc.vector.tensor_scalar(
                z[:].bitcast(i32), tsh[:].bitcast(i32), -1, 0x5F3759DF,
                Alu.mult, Alu.add,
            )
            t1 = wrk.tile([128, W], f32)
            for _ in range(3):
                nc.vector.tensor_tensor(t1[:], z[:], z[:], Alu.mult)
                nc.vector.tensor_tensor(t1[:], t1[:], pnsq[:], Alu.mult)
                nc.vector.tensor_scalar(
                    t1[:], t1[:], -0.5, 1.5, Alu.mult, Alu.add
                )
                nc.vector.tensor_tensor(z[:], z[:], t1[:], Alu.mult)
            pn = wrk.tile([128, W], f32)
            nc.vector.tensor_tensor(pn[:], pnsq[:], z[:], Alu.mult)

            # softplus(pos - neg + alpha) = ln(1 + exp(...))
            x = wrk.tile([128, OWN_CHUNKS], f32)
            nc.vector.tensor_tensor(
                x[:], pn[:, 1::2], pn[:, 0::2], Alu.subtract
            )
            e = wrk.tile([128, OWN_CHUNKS], f32)
            nc.scalar.activation(e[:], x[:], Act.Exp, bias=alpha_sb[:])
            sp = wrk.tile([128, OWN_CHUNKS], f32)
            nc.scalar.activation(sp[:], e[:], Act.Ln, bias=1.0)
            rowsum = wrk.tile([128, 1], f32)
            nc.vector.tensor_reduce(
                rowsum[:], sp[:], mybir.AxisListType.X, Alu.add
            )
            psum_out = psm.tile([1, 1], f32, name="psum_out", tag="sm")
            nc.tensor.matmul(psum_out[:], rowsum[:], onesc_sb[:])
            out_sb = wrk.tile([1, 1], f32)
            nc.scalar.activation(out_sb[:], psum_out[:], Act.Copy)
            nc.sync.dma_start(out_d.ap(), out_sb[:])

    nc.compile()
    return nc


def _get_compiled():
    global _compiled
    if _compiled is None:
        _compiled = _build()
    return _compiled


def _chunk_major_labels(lab_f32):
    # labels[j*128 + p] -> [p, j]
    n_chunks = lab_f32.shape[0] // 128
    return np.ascontiguousarray(lab_f32.reshape(n_chunks, 128).T)


def kernel(preds, labels, _trace=False):
    preds = np.ascontiguousarray(np.asarray(preds, dtype=np.float32))
    lab_f32 = np.asarray(labels, dtype=np.float32)
    assert preds.shape == (N, D) and lab_f32.shape == (N,)

    nc = _get_compiled()
    lab_cm = _chunk_major_labels(lab_f32)
    in_maps = []
    for c in range(N_CORES):
        r0, r1 = c * ROWS_PER_CORE, (c + 1) * ROWS_PER_CORE
        in_maps.append(
            {
                "preds": preds,
                "labels": lab_cm,
                "my_preds": np.ascontiguousarray(preds[r0:r1]),
                "my_labels": _chunk_major_labels(lab_f32[r0:r1]),
            }
        )

    res = bass_utils.run_bass_kernel_spmd(
        nc, in_maps, core_ids=list(range(N_CORES)), trace=_trace
    )
    global last_results
    last_results = res
    total = sum(float(res.results[c]["out"][0, 0]) for c in range(N_CORES))
    return np.float32(total / N)
